# revision 26
# baseline (speedup 1.0000x reference)
"""Trainium2 Bass kernel for nn_CholeskyConstraintLayer.

Maps x:(B,16) f32 -> rho:(B,4,4,2) f32 where rho = L L^dagger / (trace + eps),
L lower-triangular complex 4x4 built from x (softplus diagonal, raw re/im
off-diagonals).

x flat order: [d0, r10,i10, d1, r20,i20, r21,i21, d2, r30,i30, r31,i31,
r32,i32, d3]  (d* get softplus).

Device computes the 16 unique values per sample plus the 6 negated
off-diagonal imag values (22 fp16 outputs); the host only *gathers* them
into the full (4,4,2) layout (upper-triangle re is a byte-copy of lower,
diag imag is zero-fill).  I/O is fp16: in 24 els/sample (x + a host-side
pair-swapped duplicate region S used to build the conjugate products),
out 22 els/sample.  All arithmetic (softplus, products, sums, reciprocal,
normalise, negation) happens on device.

Per-sample math on device (y = x after softplus at 0,3,8,15):
  z   = S * mask,  S = x[[5,4,7, 10,9,12,11,14]], mask = (1,-1,1, 1,-1,1,-1,1)
  re21,re31,re32a = y[4:7]*y[1:4], y[9:12]*y[1:4], y[9:12]*y[4:7] (3-dots)
  re32b = y[12:14]*y[7:9];  im*a = z-slices * y-slices;  im32b = z[6:8]*y[7:9]
  dots via add-tree (el0+el1)+el2 (+ b-part for the (3,2) entry)
  diag: tensor_reduce over squares (3,5,7); trace chain + 1/t (f32)
  out = values * rcp (j=0 column folds d0 into dr = d0*rcp)

Engine split per tile: ACT softplus+squares+z, DVE products/adds/chain/
reciprocal/some norm + negations (fast tensor_scalar), Pool diag reduces +
the remaining normalise ops via scalar_tensor_tensor.  DMA via nc.sync.
"""

import numpy as np

P = 128
EPS = 1e-8
N_CORES = 8
BATCH = 1_000_000
# Tapered tile sizes (samples per partition per tile); sum*P*N_CORES >= BATCH.
F_LIST = [64, 160, 160, 160, 160, 160, 81, 32]  # sum = 977
S_CORE = P * sum(F_LIST)  # 125056
S_PAD = S_CORE * N_CORES  # 1000448

IN_W = 16   # fp16 els per sample on the way in
OUT_W = 22  # fp16 els per sample on the way out

# out slot -> rho flat-32 expansion (host): rho32[k] = out22[EXP_SRC[k]],
# EXP_SRC=-1 -> 0.  out22 layout:
# [q11,q22,q33, q00, re10,im10, re20,im20, re30,im30, re21,re31,re32,
#  im21,im31,im32, nim10,nim20,nim30, nim21,nim31,nim32]
EXP_SRC = np.full(32, -1, dtype=np.int64)
for flat, src in {
    0: 3, 10: 0, 20: 1, 30: 2,
    8: 4, 9: 5, 2: 4, 3: 16,
    16: 6, 17: 7, 4: 6, 5: 17,
    24: 8, 25: 9, 6: 8, 7: 18,
    18: 10, 19: 13, 12: 10, 13: 19,
    26: 11, 27: 14, 14: 11, 15: 20,
    28: 12, 29: 15, 22: 12, 23: 21,
}.items():
    EXP_SRC[flat] = src

_NC_CACHE = {}


def _emit(tc, x_ap, out_ap, f_list):
    import concourse.bass as bass
    import concourse.mybir as mybir
    from contextlib import ExitStack

    nc = tc.nc
    f16 = mybir.dt.float16
    f32 = mybir.dt.float32
    A = mybir.AluOpType
    ACT = mybir.ActivationFunctionType
    X = mybir.AxisListType.X

    def ap3(view3, offset, dims):
        """AP with explicit free dims [[stride,count],...] on a (p,F,W) view."""
        return bass.AP(tensor=view3.tensor, offset=view3.offset + offset,
                       ap=[list(view3.ap[0])] + [list(d) for d in dims])

    with ExitStack() as ctx:
        tp = lambda name, bufs: ctx.enter_context(tc.tile_pool(name=name, bufs=bufs))
        ipool = tp("in", 4)
        sqpool = tp("sq", 4)
        zpool = tp("z", 4)
        prpool = tp("pr", 4)
        dpool = tp("dots", 4)
        cpool = tp("chain", 4)
        opool = tp("out", 4)
        kpool = tp("const", 1)

        # constant +-1 mask for z (built once; bcast-read across samples)
        mk_t = kpool.tile([P, 8], f16, tag="mask", name="mask")
        nc.vector.memset(mk_t[:, :], 1.0)
        nc.vector.memset(mk_t[:, 1:2], -1.0)
        nc.vector.memset(mk_t[:, 4:7:2], -1.0)

        def emit_head(ti, F, s0):
            # ---- DMA in: partition p holds samples s0+p*F .. s0+(p+1)*F-1
            in_t = ipool.tile([P, F * IN_W], f16, tag="in", name=f"in{ti}")
            xin = bass.AP(tensor=x_ap.tensor, offset=(s0 * IN_W),
                          ap=[[F * IN_W, P], [1, F * IN_W]])
            nc.sync.dma_start(in_t[:, :], xin)

            v = in_t[:, :].rearrange("p (f e) -> p f e", e=IN_W)
            y = lambda a, b: v[:, :, a:b]

            # ---- ACT: softplus = Ln(Exp(x)+1) on diag slots (3,8) and
            # (0,15), in place (sq cols as exp scratch; Square later
            # overwrites all of sq)
            sq_t = sqpool.tile([P, F * 16], f16, tag="sq", name=f"sq{ti}")
            sq = sq_t[:, :].rearrange("p (f e) -> p f e", e=16)
            for off, st in ((3, 5), (0, 15)):
                src = ap3(v, off, [[IN_W, F], [st, 2]])
                tmp = ap3(sq, off, [[16, F], [st, 2]])
                nc.scalar.activation(tmp, src, ACT.Exp)
                nc.scalar.activation(src, tmp, ACT.Ln, bias=1.0)

            # ---- ACT: z = pair-swapped, sign-flipped rows for the imag
            # dots: (i20,-r20,i21 | i30,-r30,i31,-r31,i32)
            z_t = zpool.tile([P, F * 8], f16, tag="z", name=f"z{ti}")
            z = z_t[:, :].rearrange("p (f e) -> p f e", e=8)
            nc.scalar.copy(ap3(z, 0, [[8, F], [2, 2]]), ap3(v, 5, [[IN_W, F], [2, 2]]))
            nc.scalar.copy(ap3(z, 3, [[8, F], [2, 3]]), ap3(v, 10, [[IN_W, F], [2, 3]]))
            nc.scalar.mul(z[:, :, 1:2], v[:, :, 4:5], -1.0)
            nc.scalar.mul(ap3(z, 4, [[8, F], [2, 2]]), ap3(v, 9, [[IN_W, F], [2, 2]]), -1.0)

            # ---- ACT: squares of y (only 0:16 needed)
            nc.scalar.activation(sq[:, :, :], y(0, 16), ACT.Square)

            # ---- DVE: products.  prA: 6 segments x 3; prB: [re32b(2) im32b(2)]
            prA_t = prpool.tile([P, F * 18], f16, tag="prA", name=f"prA{ti}")
            prB_t = prpool.tile([P, F * 4], f16, tag="prB", name=f"prB{ti}")
            pa = prA_t[:, :].rearrange("p (f e) -> p f e", e=18)
            pb = prB_t[:, :].rearrange("p (f e) -> p f e", e=4)
            nc.gpsimd.tensor_tensor(pa[:, :, 0:3], y(4, 7), y(1, 4), op=A.mult)
            nc.gpsimd.tensor_tensor(pa[:, :, 3:6], y(9, 12), y(1, 4), op=A.mult)
            nc.vector.tensor_tensor(pa[:, :, 6:9], y(9, 12), y(4, 7), op=A.mult)
            nc.vector.tensor_tensor(pa[:, :, 9:12], z[:, :, 0:3], y(1, 4), op=A.mult)
            nc.vector.tensor_tensor(pa[:, :, 12:15], z[:, :, 3:6], y(1, 4), op=A.mult)
            nc.vector.tensor_tensor(pa[:, :, 15:18], z[:, :, 3:6], y(4, 7), op=A.mult)
            nc.vector.tensor_tensor(pb[:, :, 0:2], y(12, 14), y(7, 9), op=A.mult)
            nc.vector.tensor_tensor(pb[:, :, 2:4], z[:, :, 6:8], y(7, 9), op=A.mult)

            # ---- DVE: add-tree -> dots = [re21,re31,re32,im21,im31,im32]
            d_t = dpool.tile([P, F * 8], f16, tag="dots", name=f"d{ti}")
            dv = d_t[:, :].rearrange("p (f e) -> p f e", e=8)
            dots = dv[:, :, 0:6]
            a01 = dv[:, :, 0:6]  # reuse dots slots for the partial sum
            el = lambda k: ap3(pa, k, [[18, F], [3, 6]])
            nc.vector.tensor_tensor(a01, el(0), el(1), op=A.add)
            nc.vector.tensor_tensor(dots, a01, el(2), op=A.add)
            bsum = dv[:, :, 6:8]
            nc.vector.tensor_tensor(bsum, ap3(pb, 0, [[4, F], [2, 2]]),
                                    ap3(pb, 1, [[4, F], [2, 2]]), op=A.add)
            d32 = ap3(dv, 2, [[8, F], [3, 2]])  # dots[2], dots[5]
            nc.vector.tensor_tensor(d32, d32, bsum, op=A.add)

            # ---- DVE: diag reduces (fp16 sums, plenty for the 2e-2 budget)
            q_t = dpool.tile([P, F * 3], f16, tag="q", name=f"q{ti}")
            qv = q_t[:, :].rearrange("p (f e) -> p f e", e=3)
            nc.vector.tensor_reduce(qv[:, :, 0:1], sq[:, :, 1:4], axis=X, op=A.add)
            nc.vector.tensor_reduce(qv[:, :, 1:2], sq[:, :, 4:9], axis=X, op=A.add)
            nc.vector.tensor_reduce(qv[:, :, 2:3], sq[:, :, 9:16], axis=X, op=A.add)

            # ---- DVE: trace chain (f32): t1=q11+q22; t2=q33+sq0;
            # trE=t1+eps+t2; rcp; dr=d0*rcp
            c_t = cpool.tile([P, F * 4], f32, tag="chain", name=f"c{ti}")
            cv = c_t[:, :].rearrange("p (f e) -> p f e", e=4)
            nc.vector.tensor_tensor(cv[:, :, 0:1], qv[:, :, 0:1],
                                    qv[:, :, 1:2], op=A.add)
            nc.vector.tensor_tensor(cv[:, :, 1:2], qv[:, :, 2:3],
                                    sq[:, :, 0:1], op=A.add)
            trE = cv[:, :, 2:3]
            nc.vector.scalar_tensor_tensor(trE, cv[:, :, 0:1], float(EPS),
                                           cv[:, :, 1:2], op0=A.add, op1=A.add)
            rcp = cv[:, :, 3:4]
            nc.vector.reciprocal_approx_fast(rcp, trE)
            dr_t = dpool.tile([P, F], f16, tag="dr", name=f"dr{ti}")
            dr = dr_t[:, :].rearrange("p (f e) -> p f e", e=1)
            nc.vector.tensor_tensor(dr, v[:, :, 0:1], rcp, op=A.mult)
            return dict(ti=ti, F=F, s0=s0, v=v, y=y, qv=qv, cv=cv,
                        dots=dots, dr=dr)

        def emit_tail(st):
            ti, F, s0, y = st["ti"], st["F"], st["s0"], st["y"]
            qv, cv, dots, dr = st["qv"], st["cv"], st["dots"], st["dr"]
            # ---- normalise into out tile
            out_t = opool.tile([P, F * OUT_W], f16, tag="out", name=f"o{ti}")
            ov = out_t[:, :].rearrange("p (f e) -> p f e", e=OUT_W)
            rcp_b = lambda k: ap3(cv, 3, [[4, F], [0, k]])
            dr_b = lambda k: ap3(dr, 0, [[1, F], [0, k]])

            # Pool: diag * rcp, (q00 + j0 pairs) * dr, dots * rcp
            nc.gpsimd.tensor_tensor(ov[:, :, 0:3], qv, rcp_b(3), op=A.mult)
            nc.gpsimd.tensor_tensor(ov[:, :, 3:6], y(0, 3), dr_b(3), op=A.mult)
            nc.gpsimd.tensor_tensor(ov[:, :, 6:8], y(4, 6), dr_b(2), op=A.mult)
            nc.gpsimd.tensor_tensor(ov[:, :, 8:10], y(9, 11), dr_b(2), op=A.mult)
            nc.gpsimd.tensor_tensor(ov[:, :, 10:16], dots, rcp_b(6), op=A.mult)
            # ACT: negated imag copies (keeps the DVE queue off the tail)
            nc.scalar.mul(ov[:, :, 16:19],
                          ap3(ov, 5, [[OUT_W, F], [2, 3]]), -1.0)
            nc.scalar.mul(ov[:, :, 19:22], ov[:, :, 13:16], -1.0)

            # ---- DMA out
            odst = bass.AP(tensor=out_ap.tensor, offset=(s0 * OUT_W),
                           ap=[[F * OUT_W, P], [1, F * OUT_W]])
            nc.sync.dma_start(odst, out_t[:, :])

        # Software pipeline: emit tile t's tail AFTER tile t+1's head so the
        # in-order engine queues never head-of-line block on the cross-engine
        # tail (norm -> negs -> dma-out) of the previous tile.
        s0 = 0
        pending = None
        for ti, F in enumerate(f_list):
            st = emit_head(ti, F, s0)
            if pending is not None:
                emit_tail(pending)
            pending = st
            s0 += P * F
        emit_tail(pending)


def _patch_act_tables():
    """Force every ACT function onto one table set so the table-load pass
    emits a single load (Softplus/Square/Copy must be co-resident on HW for
    this to be numerically safe -- verified by the harness rel-err check)."""
    import concourse.bacc as bacc
    from concourse.hw_specs import get_activation_tables as _orig

    if getattr(bacc, "_act_tables_patched", False):
        return

    def _patched(arch):
        t = _orig(arch)
        return {k: (v if k == "natural_log_exp_and_others" else set())
                for k, v in t.items()}

    bacc.get_activation_tables = _patched
    bacc._act_tables_patched = True


def _build_nc(f_list):
    import concourse.bacc as bacc
    import concourse.mybir as mybir
    import concourse.tile as tile

    _patch_act_tables()

    key = tuple(f_list)
    if key in _NC_CACHE:
        return _NC_CACHE[key]
    S = P * sum(f_list)
    nc = bacc.Bacc("TRN2", target_bir_lowering=False, debug=False)
    x = nc.dram_tensor("x", (S, IN_W), mybir.dt.float16, kind="ExternalInput")
    out = nc.dram_tensor("out", (S, OUT_W), mybir.dt.float16, kind="ExternalOutput")
    with tile.TileContext(nc) as tc:
        with nc.allow_low_precision(reason="fp16 pipeline, rel-err budget 2e-2"):
            _emit(tc, x.ap(), out.ap(), f_list)
    nc.compile()
    _NC_CACHE[key] = nc
    return nc


def kernel(x, _trace=False):
    from concourse.bass_utils import run_bass_kernel_spmd

    x = np.ascontiguousarray(np.asarray(x, dtype=np.float32))
    B = x.shape[0]
    assert x.shape == (B, 16) and B <= S_PAD
    # staging: pad, append the pair-swapped duplicate region, cast to fp16
    xp = np.zeros((S_PAD, IN_W), dtype=np.float16)
    xp[:B] = x
    shards = xp.reshape(N_CORES, S_CORE, IN_W)
    nc = _build_nc(F_LIST)
    in_maps = [{"x": np.ascontiguousarray(shards[i])} for i in range(N_CORES)]
    res = run_bass_kernel_spmd(nc, in_maps, core_ids=list(range(N_CORES)),
                               trace=_trace)
    out22 = np.concatenate([r["out"].reshape(S_CORE, OUT_W) for r in res.results],
                           axis=0)[:B]
    # host: pure gather/zero-fill expansion to the full (4,4,2) layout
    out32 = np.zeros((B, 32), dtype=np.float32)
    used = EXP_SRC >= 0
    out32[:, used] = out22[:, EXP_SRC[used]].astype(np.float32)
    result = out32.reshape(B, 4, 4, 2)
    if _trace:
        return result, res
    return result


# revision 27
# speedup vs baseline: 1.0062x; 1.0062x over previous
"""Trainium2 Bass kernel for nn_CholeskyConstraintLayer.

Maps x:(B,16) f32 -> rho:(B,4,4,2) f32 where rho = L L^dagger / (trace + eps),
L lower-triangular complex 4x4 built from x (softplus diagonal, raw re/im
off-diagonals).

x flat order: [d0, r10,i10, d1, r20,i20, r21,i21, d2, r30,i30, r31,i31,
r32,i32, d3]  (d* get softplus).

Device computes the 16 unique values per sample plus the 6 negated
off-diagonal imag values (22 fp16 outputs); the host only *gathers* them
into the full (4,4,2) layout (upper-triangle re is a byte-copy of lower,
diag imag is zero-fill).  I/O is fp16: in 24 els/sample (x + a host-side
pair-swapped duplicate region S used to build the conjugate products),
out 22 els/sample.  All arithmetic (softplus, products, sums, reciprocal,
normalise, negation) happens on device.

Per-sample math on device (y = x after softplus at 0,3,8,15):
  z   = S * mask,  S = x[[5,4,7, 10,9,12,11,14]], mask = (1,-1,1, 1,-1,1,-1,1)
  re21,re31,re32a = y[4:7]*y[1:4], y[9:12]*y[1:4], y[9:12]*y[4:7] (3-dots)
  re32b = y[12:14]*y[7:9];  im*a = z-slices * y-slices;  im32b = z[6:8]*y[7:9]
  dots via add-tree (el0+el1)+el2 (+ b-part for the (3,2) entry)
  diag: tensor_reduce over squares (3,5,7); trace chain + 1/t (f32)
  out = values * rcp (j=0 column folds d0 into dr = d0*rcp)

Engine split per tile: ACT softplus+squares+z, DVE products/adds/chain/
reciprocal/some norm + negations (fast tensor_scalar), Pool diag reduces +
the remaining normalise ops via scalar_tensor_tensor.  DMA via nc.sync.
"""

import numpy as np

P = 128
EPS = 1e-8
N_CORES = 8
BATCH = 1_000_000
# Tapered tile sizes (samples per partition per tile); sum*P*N_CORES >= BATCH.
F_LIST = [64, 160, 160, 160, 160, 160, 81, 32]  # sum = 977
S_CORE = P * sum(F_LIST)  # 125056
S_PAD = S_CORE * N_CORES  # 1000448

IN_W = 16   # fp16 els per sample on the way in
OUT_W = 22  # fp16 els per sample on the way out

# out slot -> rho flat-32 expansion (host): rho32[k] = out22[EXP_SRC[k]],
# EXP_SRC=-1 -> 0.  out22 layout:
# [q11,q22,q33, q00, re10,im10, re20,im20, re30,im30, re21,re31,re32,
#  im21,im31,im32, nim10,nim20,nim30, nim21,nim31,nim32]
EXP_SRC = np.full(32, -1, dtype=np.int64)
for flat, src in {
    0: 3, 10: 0, 20: 1, 30: 2,
    8: 4, 9: 5, 2: 4, 3: 16,
    16: 6, 17: 7, 4: 6, 5: 17,
    24: 8, 25: 9, 6: 8, 7: 18,
    18: 10, 19: 13, 12: 10, 13: 19,
    26: 11, 27: 14, 14: 11, 15: 20,
    28: 12, 29: 15, 22: 12, 23: 21,
}.items():
    EXP_SRC[flat] = src

_NC_CACHE = {}


def _emit(tc, x_ap, out_ap, f_list):
    import concourse.bass as bass
    import concourse.mybir as mybir
    from contextlib import ExitStack

    nc = tc.nc
    f16 = mybir.dt.float16
    f32 = mybir.dt.float32
    A = mybir.AluOpType
    ACT = mybir.ActivationFunctionType
    X = mybir.AxisListType.X

    def ap3(view3, offset, dims):
        """AP with explicit free dims [[stride,count],...] on a (p,F,W) view."""
        return bass.AP(tensor=view3.tensor, offset=view3.offset + offset,
                       ap=[list(view3.ap[0])] + [list(d) for d in dims])

    with ExitStack() as ctx:
        tp = lambda name, bufs: ctx.enter_context(tc.tile_pool(name=name, bufs=bufs))
        ipool = tp("in", 4)
        sqpool = tp("sq", 4)
        zpool = tp("z", 4)
        prpool = tp("pr", 4)
        dpool = tp("dots", 4)
        cpool = tp("chain", 4)
        opool = tp("out", 4)
        kpool = tp("const", 1)

        # constant +-1 mask for z (built once; bcast-read across samples)
        mk_t = kpool.tile([P, 8], f16, tag="mask", name="mask")
        nc.vector.memset(mk_t[:, :], 1.0)
        nc.vector.memset(mk_t[:, 1:2], -1.0)
        nc.vector.memset(mk_t[:, 4:7:2], -1.0)

        def emit_head(ti, F, s0):
            # ---- DMA in: partition p holds samples s0+p*F .. s0+(p+1)*F-1
            in_t = ipool.tile([P, F * IN_W], f16, tag="in", name=f"in{ti}")
            xin = bass.AP(tensor=x_ap.tensor, offset=(s0 * IN_W),
                          ap=[[F * IN_W, P], [1, F * IN_W]])
            nc.sync.dma_start(in_t[:, :], xin)

            v = in_t[:, :].rearrange("p (f e) -> p f e", e=IN_W)
            y = lambda a, b: v[:, :, a:b]

            # ---- ACT: softplus = Ln(Exp(x)+1) on diag slots (3,8) and
            # (0,15), in place (sq cols as exp scratch; Square later
            # overwrites all of sq)
            sq_t = sqpool.tile([P, F * 16], f16, tag="sq", name=f"sq{ti}")
            sq = sq_t[:, :].rearrange("p (f e) -> p f e", e=16)
            for off, st in ((3, 5), (0, 15)):
                src = ap3(v, off, [[IN_W, F], [st, 2]])
                tmp = ap3(sq, off, [[16, F], [st, 2]])
                nc.scalar.activation(tmp, src, ACT.Exp)
                nc.scalar.activation(src, tmp, ACT.Ln, bias=1.0)

            # ---- ACT: z = pair-swapped, sign-flipped rows for the imag
            # dots: (i20,-r20,i21 | i30,-r30,i31,-r31,i32)
            z_t = zpool.tile([P, F * 8], f16, tag="z", name=f"z{ti}")
            z = z_t[:, :].rearrange("p (f e) -> p f e", e=8)
            nc.scalar.copy(ap3(z, 0, [[8, F], [2, 2]]), ap3(v, 5, [[IN_W, F], [2, 2]]))
            nc.scalar.copy(ap3(z, 3, [[8, F], [2, 3]]), ap3(v, 10, [[IN_W, F], [2, 3]]))
            nc.scalar.mul(z[:, :, 1:2], v[:, :, 4:5], -1.0)
            nc.scalar.mul(ap3(z, 4, [[8, F], [2, 2]]), ap3(v, 9, [[IN_W, F], [2, 2]]), -1.0)

            # ---- ACT: squares of y (only 0:16 needed)
            nc.scalar.activation(sq[:, :, :], y(0, 16), ACT.Square)

            # ---- DVE: products.  prA: 6 segments x 3; prB: [re32b(2) im32b(2)]
            prA_t = prpool.tile([P, F * 18], f16, tag="prA", name=f"prA{ti}")
            prB_t = prpool.tile([P, F * 4], f16, tag="prB", name=f"prB{ti}")
            pa = prA_t[:, :].rearrange("p (f e) -> p f e", e=18)
            pb = prB_t[:, :].rearrange("p (f e) -> p f e", e=4)
            nc.gpsimd.tensor_tensor(pa[:, :, 0:3], y(4, 7), y(1, 4), op=A.mult)
            nc.gpsimd.tensor_tensor(pa[:, :, 3:6], y(9, 12), y(1, 4), op=A.mult)
            nc.vector.tensor_tensor(pa[:, :, 6:9], y(9, 12), y(4, 7), op=A.mult)
            nc.vector.tensor_tensor(pa[:, :, 9:12], z[:, :, 0:3], y(1, 4), op=A.mult)
            nc.vector.tensor_tensor(pa[:, :, 12:15], z[:, :, 3:6], y(1, 4), op=A.mult)
            nc.vector.tensor_tensor(pa[:, :, 15:18], z[:, :, 3:6], y(4, 7), op=A.mult)
            nc.vector.tensor_tensor(pb[:, :, 0:2], y(12, 14), y(7, 9), op=A.mult)
            nc.vector.tensor_tensor(pb[:, :, 2:4], z[:, :, 6:8], y(7, 9), op=A.mult)

            # ---- DVE: add-tree -> dots = [re21,re31,re32,im21,im31,im32]
            d_t = dpool.tile([P, F * 8], f16, tag="dots", name=f"d{ti}")
            dv = d_t[:, :].rearrange("p (f e) -> p f e", e=8)
            dots = dv[:, :, 0:6]
            a01 = dv[:, :, 0:6]  # reuse dots slots for the partial sum
            el = lambda k: ap3(pa, k, [[18, F], [3, 6]])
            nc.vector.tensor_tensor(a01, el(0), el(1), op=A.add)
            nc.vector.tensor_tensor(dots, a01, el(2), op=A.add)
            bsum = dv[:, :, 6:8]
            nc.vector.tensor_tensor(bsum, ap3(pb, 0, [[4, F], [2, 2]]),
                                    ap3(pb, 1, [[4, F], [2, 2]]), op=A.add)
            d32 = ap3(dv, 2, [[8, F], [3, 2]])  # dots[2], dots[5]
            nc.vector.tensor_tensor(d32, d32, bsum, op=A.add)

            # ---- DVE: diag reduces (fp16 sums, plenty for the 2e-2 budget)
            q_t = dpool.tile([P, F * 3], f16, tag="q", name=f"q{ti}")
            qv = q_t[:, :].rearrange("p (f e) -> p f e", e=3)
            nc.vector.tensor_reduce(qv[:, :, 0:1], sq[:, :, 1:4], axis=X, op=A.add)
            nc.vector.tensor_reduce(qv[:, :, 1:2], sq[:, :, 4:9], axis=X, op=A.add)
            nc.vector.tensor_reduce(qv[:, :, 2:3], sq[:, :, 9:16], axis=X, op=A.add)

            # ---- DVE: trace chain (f32): t1=q11+q22; t2=q33+sq0;
            # trE=t1+eps+t2; rcp; dr=d0*rcp
            c_t = cpool.tile([P, F * 4], f32, tag="chain", name=f"c{ti}")
            cv = c_t[:, :].rearrange("p (f e) -> p f e", e=4)
            nc.vector.tensor_tensor(cv[:, :, 0:1], qv[:, :, 0:1],
                                    qv[:, :, 1:2], op=A.add)
            nc.vector.tensor_tensor(cv[:, :, 1:2], qv[:, :, 2:3],
                                    sq[:, :, 0:1], op=A.add)
            trE = cv[:, :, 2:3]
            nc.vector.scalar_tensor_tensor(trE, cv[:, :, 0:1], float(EPS),
                                           cv[:, :, 1:2], op0=A.add, op1=A.add)
            rcp = cv[:, :, 3:4]
            nc.vector.reciprocal_approx_fast(rcp, trE)
            dr_t = dpool.tile([P, F], f16, tag="dr", name=f"dr{ti}")
            dr = dr_t[:, :].rearrange("p (f e) -> p f e", e=1)
            nc.vector.tensor_tensor(dr, v[:, :, 0:1], rcp, op=A.mult)
            return dict(ti=ti, F=F, s0=s0, v=v, y=y, qv=qv, cv=cv,
                        dots=dots, dr=dr, tail=(ti >= len(f_list) - 2))

        def emit_tail(st):
            ti, F, s0, y = st["ti"], st["F"], st["s0"], st["y"]
            qv, cv, dots, dr = st["qv"], st["cv"], st["dots"], st["dr"]
            # ---- normalise into out tile
            out_t = opool.tile([P, F * OUT_W], f16, tag="out", name=f"o{ti}")
            ov = out_t[:, :].rearrange("p (f e) -> p f e", e=OUT_W)
            rcp_b = lambda k: ap3(cv, 3, [[4, F], [0, k]])
            dr_b = lambda k: ap3(dr, 0, [[1, F], [0, k]])

            # Norm: Pool mid-stream; DVE for the tail tiles (Pool's serial
            # queue drain would otherwise dominate the kernel tail)
            ne = nc.vector if st["tail"] else nc.gpsimd
            ne.tensor_tensor(ov[:, :, 0:3], qv, rcp_b(3), op=A.mult)
            ne.tensor_tensor(ov[:, :, 3:6], y(0, 3), dr_b(3), op=A.mult)
            ne.tensor_tensor(ov[:, :, 6:8], y(4, 6), dr_b(2), op=A.mult)
            ne.tensor_tensor(ov[:, :, 8:10], y(9, 11), dr_b(2), op=A.mult)
            ne.tensor_tensor(ov[:, :, 10:16], dots, rcp_b(6), op=A.mult)
            # negated imag copies: ACT mid-stream, DVE on tail tiles
            if st["tail"]:
                nc.vector.tensor_scalar_mul(ov[:, :, 16:19],
                                            ap3(ov, 5, [[OUT_W, F], [2, 3]]), -1.0)
                nc.vector.tensor_scalar_mul(ov[:, :, 19:22], ov[:, :, 13:16], -1.0)
            else:
                nc.scalar.mul(ov[:, :, 16:19],
                              ap3(ov, 5, [[OUT_W, F], [2, 3]]), -1.0)
                nc.scalar.mul(ov[:, :, 19:22], ov[:, :, 13:16], -1.0)

            # ---- DMA out
            odst = bass.AP(tensor=out_ap.tensor, offset=(s0 * OUT_W),
                           ap=[[F * OUT_W, P], [1, F * OUT_W]])
            nc.sync.dma_start(odst, out_t[:, :])

        # Software pipeline: emit tile t's tail AFTER tile t+1's head so the
        # in-order engine queues never head-of-line block on the cross-engine
        # tail (norm -> negs -> dma-out) of the previous tile.
        s0 = 0
        pending = None
        for ti, F in enumerate(f_list):
            st = emit_head(ti, F, s0)
            if pending is not None:
                emit_tail(pending)
            pending = st
            s0 += P * F
        emit_tail(pending)


def _patch_act_tables():
    """Force every ACT function onto one table set so the table-load pass
    emits a single load (Softplus/Square/Copy must be co-resident on HW for
    this to be numerically safe -- verified by the harness rel-err check)."""
    import concourse.bacc as bacc
    from concourse.hw_specs import get_activation_tables as _orig

    if getattr(bacc, "_act_tables_patched", False):
        return

    def _patched(arch):
        t = _orig(arch)
        return {k: (v if k == "natural_log_exp_and_others" else set())
                for k, v in t.items()}

    bacc.get_activation_tables = _patched
    bacc._act_tables_patched = True


def _build_nc(f_list):
    import concourse.bacc as bacc
    import concourse.mybir as mybir
    import concourse.tile as tile

    _patch_act_tables()

    key = tuple(f_list)
    if key in _NC_CACHE:
        return _NC_CACHE[key]
    S = P * sum(f_list)
    nc = bacc.Bacc("TRN2", target_bir_lowering=False, debug=False)
    x = nc.dram_tensor("x", (S, IN_W), mybir.dt.float16, kind="ExternalInput")
    out = nc.dram_tensor("out", (S, OUT_W), mybir.dt.float16, kind="ExternalOutput")
    with tile.TileContext(nc) as tc:
        with nc.allow_low_precision(reason="fp16 pipeline, rel-err budget 2e-2"):
            _emit(tc, x.ap(), out.ap(), f_list)
    nc.compile()
    _NC_CACHE[key] = nc
    return nc


def kernel(x, _trace=False):
    from concourse.bass_utils import run_bass_kernel_spmd

    x = np.ascontiguousarray(np.asarray(x, dtype=np.float32))
    B = x.shape[0]
    assert x.shape == (B, 16) and B <= S_PAD
    # staging: pad, append the pair-swapped duplicate region, cast to fp16
    xp = np.zeros((S_PAD, IN_W), dtype=np.float16)
    xp[:B] = x
    shards = xp.reshape(N_CORES, S_CORE, IN_W)
    nc = _build_nc(F_LIST)
    in_maps = [{"x": np.ascontiguousarray(shards[i])} for i in range(N_CORES)]
    res = run_bass_kernel_spmd(nc, in_maps, core_ids=list(range(N_CORES)),
                               trace=_trace)
    out22 = np.concatenate([r["out"].reshape(S_CORE, OUT_W) for r in res.results],
                           axis=0)[:B]
    # host: pure gather/zero-fill expansion to the full (4,4,2) layout
    out32 = np.zeros((B, 32), dtype=np.float32)
    used = EXP_SRC >= 0
    out32[:, used] = out22[:, EXP_SRC[used]].astype(np.float32)
    result = out32.reshape(B, 4, 4, 2)
    if _trace:
        return result, res
    return result
